# revision 4
# baseline (speedup 1.0000x reference)
"""GAT attention head (single head) distributed across 8 TRN2 NeuronCores.

Math (reference):
    sf   = seq @ W1                        # [N, O]
    f1   = sf @ a1 + b1                    # [N, 1]
    f2   = sf @ a2 + b2                    # [N, 1]
    lg   = f1 + f2.T                       # [N, N]
    co   = softmax(leaky_relu(lg, 0.2) + bias, axis=-1)
    out  = elu(co @ sf)                    # [N, O]

Key algebraic transform: with S = f1[r] + f2[n],
    leaky_relu(S, 0.2) = 0.2*S + 0.8*relu(S)
and softmax over n is invariant to adding any g(r), so the 0.2*f1[r] term is
dropped.  Each core therefore computes (transposed layout: n on partitions,
r on the free dim):
    x[n, r]  = 0.8*relu(f1[r] + f2[n]) + biasT[n, r]        (PE accumulates)
    e[n, r]  = exp(x + 0.2*f2[n])                           (one ACT pass)
    valsT    = [sf | 1s].T @ e   -> [O+1, R] rows 0..O-1 = unnormalized vals,
                                    row O = softmax denominators
    out      = elu(valsT[:O] / valsT[O])

Sharding: rows r are sharded across 8 cores (1024 each).  The host supplies
per-core transposed shards (seqT, biasT) so all device DMA is contiguous.
seq_fts is computed per-shard and AllGather'ed (together with f2) so every
core has the full [N, O] seq_fts for the column dimension.
"""

import sys

sys.path.insert(0, "/opt/trn_rl_repo")

import numpy as np

import concourse.bacc as bacc
import concourse.bass as bass
import concourse.mybir as mybir
import concourse.tile as tile
from concourse.bass_utils import run_bass_kernel_spmd

F32 = mybir.dt.float32
BF16 = mybir.dt.bfloat16
ADD = mybir.AluOpType.add
MAX = mybir.AluOpType.max
MIN = mybir.AluOpType.min
MULT = mybir.AluOpType.mult
EXP = mybir.ActivationFunctionType.Exp
COPY = mybir.ActivationFunctionType.Copy

M = 8          # cores
N = 8192       # nodes (columns of the attention matrix)
R = N // M     # rows per core (1024)
F_IN = 256
O = 64
P = 128        # partitions
NCH = N // P   # n-chunks per core (64)
RB = R // P    # row-blocks per core (8)
H = 512        # matmul free-dim half (PSUM bank limit)

_CACHED = {}


def build_nc(n=N, r=R, bt_bufs=14):
    nch = n // P
    rb = r // P
    hs = [slice(i * H, min((i + 1) * H, r)) for i in range((r + H - 1) // H)]

    nc = bacc.Bacc(
        "TRN2",
        target_bir_lowering=False,
        debug=False,
        enable_asserts=True,
        num_devices=M,
    )

    seqT_d = nc.dram_tensor("seqT", [F_IN, r], F32, kind="ExternalInput")
    biasT_d = nc.dram_tensor("biasT", [n, r], F32, kind="ExternalInput")
    W1_d = nc.dram_tensor("W1", [F_IN, O], F32, kind="ExternalInput")
    a1_d = nc.dram_tensor("a1", [O, 1], F32, kind="ExternalInput")
    a2_d = nc.dram_tensor("a2", [O, 1], F32, kind="ExternalInput")
    b1_d = nc.dram_tensor("b1", [1, 1], F32, kind="ExternalInput")
    b2_d = nc.dram_tensor("b2", [P, 1], F32, kind="ExternalInput")
    id10_d = nc.dram_tensor("id10", [P, P], F32, kind="ExternalInput")
    id08_d = nc.dram_tensor("id08", [P, P], F32, kind="ExternalInput")
    onesp_d = nc.dram_tensor("onesp", [1, P], F32, kind="ExternalInput")
    oneso_d = nc.dram_tensor("oneso", [1, O], F32, kind="ExternalInput")
    out_d = nc.dram_tensor("out", [O, r], F32, kind="ExternalOutput")

    with tile.TileContext(nc) as tc:
        with (
            tc.tile_pool(name="const", bufs=1) as cp,
            tc.tile_pool(name="bt", bufs=bt_bufs) as btp,
            tc.tile_pool(name="g", bufs=3) as gp,
            tc.tile_pool(name="e", bufs=3) as ep,
            tc.tile_pool(name="small", bufs=2) as smp,
            tc.tile_pool(name="xp", bufs=2, space="PSUM") as xp,
            tc.tile_pool(name="vp", bufs=1, space="PSUM") as vp,
            tc.tile_pool(name="sp", bufs=1, space="PSUM") as sp,
            tc.tile_pool(name="dram", bufs=1, space="DRAM") as dramp,
        ):
            # ---- constants in ----
            id10 = cp.tile([P, P], F32)
            nc.sync.dma_start(id10[:], id10_d.ap())
            id08 = cp.tile([P, P], F32)
            nc.sync.dma_start(id08[:], id08_d.ap())
            onesp = cp.tile([1, P], F32)
            nc.sync.dma_start(onesp[:], onesp_d.ap())
            oneso = cp.tile([1, O], F32)
            nc.sync.dma_start(oneso[:], oneso_d.ap())
            w1a = cp.tile([P, O], F32)
            nc.sync.dma_start(w1a[:], W1_d.ap()[0:P, :])
            w1b = cp.tile([P, O], F32)
            nc.sync.dma_start(w1b[:], W1_d.ap()[P:F_IN, :])
            a1s = cp.tile([O, 1], F32)
            nc.sync.dma_start(a1s[:], a1_d.ap())
            a2s = cp.tile([O, 1], F32)
            nc.sync.dma_start(a2s[:], a2_d.ap())
            b1s = cp.tile([1, 1], F32)
            nc.sync.dma_start(b1s[:], b1_d.ap())
            b2s = cp.tile([P, 1], F32)
            nc.sync.dma_start(b2s[:], b2_d.ap())
            seqta = cp.tile([P, r], F32)
            nc.sync.dma_start(seqta[:], seqT_d.ap()[0:P, :])
            seqtb = cp.tile([P, r], F32)
            nc.sync.dma_start(seqtb[:], seqT_d.ap()[P:F_IN, :])

            # ---- local shard seq_fts^T : [O, r] = W1.T @ seqT ----
            sft_ps = sp.tile([O, r], F32, tag="scratch")
            for sl in hs:
                nc.tensor.matmul(sft_ps[:, sl], w1a[:], seqta[:, sl], start=True, stop=False)
            for sl in hs:
                nc.tensor.matmul(sft_ps[:, sl], w1b[:], seqtb[:, sl], start=False, stop=True)
            sft = cp.tile([O, r], F32)
            nc.scalar.activation(sft[:], sft_ps[:], COPY)

            # ---- f1 (own rows), F1B broadcast tile ----
            f1_ps = sp.tile([1, r], F32, tag="scratch")
            for sl in hs:
                nc.tensor.matmul(f1_ps[:, sl], a1s[:], sft[:, sl], start=True, stop=True)
            f1row = cp.tile([1, r], F32)
            nc.vector.tensor_scalar(f1row[:], f1_ps[:], b1s[:], None, op0=ADD)

            f1b_ps = sp.tile([P, r], F32, tag="scratch")
            for sl in hs:
                nc.tensor.matmul(f1b_ps[:, sl], onesp[:], f1row[:, sl], start=True, stop=True)
            f1b = cp.tile([P, r], F32)
            nc.scalar.activation(f1b[:], f1b_ps[:], COPY)

            # ---- f2 of own rows: per row-block [O,P] slice of sft, matmul a2 ----
            f2_ps = sp.tile([P, rb], F32, tag="scratch")
            for j in range(rb):
                nc.tensor.matmul(
                    f2_ps[:, j : j + 1],
                    sft[:, j * P : (j + 1) * P],
                    a2s[:],
                    start=True,
                    stop=True,
                )
            f2own = cp.tile([P, rb], F32)
            nc.vector.tensor_scalar(f2own[:], f2_ps[:], b2s[:], None, op0=ADD)

            # ---- sf natural ([r, O]) via PE transposes, into AllGather input ----
            agin = dramp.tile([r, O + 1], F32)
            agout = dramp.tile([n, O + 1], F32)
            for j in range(rb):
                t_ps = sp.tile([P, O], F32, tag="scratch")
                nc.tensor.transpose(t_ps[:], sft[:, j * P : (j + 1) * P], id10[0:O, 0:O])
                t_sb = smp.tile([P, O], F32)
                nc.scalar.activation(t_sb[:], t_ps[:], COPY)
                nc.sync.dma_start(agin[j * P : (j + 1) * P, 0:O], t_sb[:])
            nc.sync.dma_start(
                agin[:].rearrange("(b p) o -> p b o", p=P)[:, :, O : O + 1],
                f2own[:].rearrange("p (b one) -> p b one", one=1),
            )

            nc.gpsimd.collective_compute(
                "AllGather",
                mybir.AluOpType.bypass,
                replica_groups=[list(range(M))],
                ins=[agin[:].opt()],
                outs=[agout[:].opt()],
            )

            # ---- unpack gathered: sfaug (bf16, [P, nch*(O+1)]) and f2col ----
            sfall = cp.tile([P, nch * O], F32)
            nc.sync.dma_start(
                sfall[:].rearrange("p (c o) -> p c o", o=O),
                agout[:].rearrange("(c p) o -> p c o", p=P)[:, :, 0:O],
            )
            sfaug = cp.tile([P, nch * (O + 1)], BF16)
            nc.vector.tensor_copy(
                sfaug[:].rearrange("p (c o) -> p c o", o=O + 1)[:, :, 0:O],
                sfall[:].rearrange("p (c o) -> p c o", o=O),
            )
            nc.vector.memset(
                sfaug[:].rearrange("p (c o) -> p c o", o=O + 1)[:, :, O : O + 1], 1.0
            )
            f2col = cp.tile([P, nch], F32)
            nc.sync.dma_start(
                f2col[:].rearrange("p (c one) -> p c one", one=1),
                agout[:].rearrange("(c p) o -> p c o", p=P)[:, :, O : O + 1],
            )
            f2col02 = cp.tile([P, nch], F32)
            nc.vector.tensor_scalar(f2col02[:], f2col[:], 0.2, None, op0=MULT)

            # ---- main loop over n-chunks ----
            vals = vp.tile([O + 1, r], F32)
            saug_r = sfaug[:].rearrange("p (c o) -> p c o", o=O + 1)
            for c in range(nch):
                bt = btp.tile([P, r], F32)
                nc.sync.dma_start(bt[:], biasT_d.ap()[c * P : (c + 1) * P, :])

                g = gp.tile([P, r], F32)
                nc.vector.tensor_scalar(
                    g[:], f1b[:], f2col[:, c : c + 1], 0.0, op0=ADD, op1=MAX
                )

                x = xp.tile([P, r], F32)
                for sl in hs:
                    nc.tensor.matmul(x[:, sl], id08[:], g[:, sl], start=True, stop=False)
                for sl in hs:
                    nc.tensor.matmul(x[:, sl], id10[:], bt[:, sl], start=False, stop=True)

                e = ep.tile([P, r], BF16)
                nc.scalar.activation(e[:], x[:], EXP, bias=f2col02[:, c : c + 1])

                for sl in hs:
                    nc.tensor.matmul(
                        vals[:, sl],
                        saug_r[:, c, :],
                        e[:, sl],
                        start=(c == 0),
                        stop=(c == nch - 1),
                    )

            # ---- epilogue: divide by row sums, elu, out ----
            recip = cp.tile([1, r], F32)
            nc.vector.reciprocal(recip[:], vals[O : O + 1, :])
            rb_ps = sp.tile([O, r], F32, tag="scratch")
            for sl in hs:
                nc.tensor.matmul(rb_ps[:, sl], oneso[:], recip[:, sl], start=True, stop=True)
            vals_sb = cp.tile([O, r], F32)
            nc.scalar.activation(vals_sb[:], vals[0:O, :], COPY)
            vn = cp.tile([O, r], F32)
            nc.vector.tensor_mul(vn[:], vals_sb[:], rb_ps[:])
            # elu(x) = (relu(x) - 1) + exp(min(x, 0))
            p2 = cp.tile([O, r], F32)
            nc.vector.tensor_scalar(p2[:], vn[:], 0.0, -1.0, op0=MAX, op1=ADD)
            mn = cp.tile([O, r], F32)
            nc.vector.tensor_scalar(mn[:], vn[:], 0.0, None, op0=MIN)
            em = cp.tile([O, r], F32)
            nc.scalar.activation(em[:], mn[:], EXP)
            outT = cp.tile([O, r], F32)
            nc.vector.tensor_add(outT[:], p2[:], em[:])
            nc.sync.dma_start(out_d.ap(), outT[:])

    nc.compile()
    return nc


def get_nc():
    if "nc" not in _CACHED:
        _CACHED["nc"] = build_nc()
    return _CACHED["nc"]


def make_in_maps(seq, bias_mat, W1, a1, b1, a2, b2, n=N, r=R):
    m = n // r
    seq2 = np.asarray(seq, dtype=np.float32).reshape(n, F_IN)
    bias2 = np.asarray(bias_mat, dtype=np.float32).reshape(n, n)
    common = {
        "W1": np.asarray(W1, np.float32).reshape(F_IN, O),
        "a1": np.asarray(a1, np.float32).reshape(O, 1),
        "a2": np.asarray(a2, np.float32).reshape(O, 1),
        "b1": np.asarray(b1, np.float32).reshape(1, 1),
        "b2": np.full((P, 1), np.float32(np.asarray(b2).reshape(())), np.float32),
        "id10": np.eye(P, dtype=np.float32),
        "id08": (0.8 * np.eye(P)).astype(np.float32),
        "onesp": np.ones((1, P), np.float32),
        "oneso": np.ones((1, O), np.float32),
    }
    in_maps = []
    for i in range(m):
        rows = slice(i * r, (i + 1) * r)
        in_maps.append(
            dict(
                common,
                seqT=np.ascontiguousarray(seq2[rows, :].T),
                biasT=np.ascontiguousarray(bias2[rows, :].T),
            )
        )
    return in_maps


def kernel(seq, bias_mat, W1, a1, b1, a2, b2):
    nc = get_nc()
    in_maps = make_in_maps(seq, bias_mat, W1, a1, b1, a2, b2)
    res = run_bass_kernel_spmd(nc, in_maps, core_ids=list(range(M)))
    outs = [res.results[i]["out"] for i in range(M)]
    full = np.concatenate([o.T for o in outs], axis=0)  # [N, O]
    return full.reshape(1, N, O).astype(np.float32)


if __name__ == "__main__":
    rng = np.random.default_rng(0)
    seq = rng.standard_normal((1, N, F_IN), dtype=np.float32)
    bias = np.zeros((1, N, N), np.float32)
    W1 = (rng.standard_normal((F_IN, O)) * 0.05).astype(np.float32)
    a1 = (rng.standard_normal((O, 1)) * 0.05).astype(np.float32)
    a2 = (rng.standard_normal((O, 1)) * 0.05).astype(np.float32)
    b1 = np.zeros((1,), np.float32)
    b2 = np.zeros((1,), np.float32)
    out = kernel(seq=seq, bias_mat=bias, W1=W1, a1=a1, b1=b1, a2=a2, b2=b2)
    print(out.shape, out.dtype)


# revision 6
# speedup vs baseline: 1.2827x; 1.2827x over previous
"""GAT attention head (single head) distributed across 8 TRN2 NeuronCores.

Math (reference):
    sf   = seq @ W1                        # [N, O]
    f1   = sf @ a1 + b1                    # [N, 1]
    f2   = sf @ a2 + b2                    # [N, 1]
    lg   = f1 + f2.T                       # [N, N]
    co   = softmax(leaky_relu(lg, 0.2) + bias, axis=-1)
    out  = elu(co @ sf)                    # [N, O]

Key algebraic transform: with S = f1[r] + f2[n],
    leaky_relu(S, 0.2) = 0.2*S + 0.8*relu(S)
and softmax over n is invariant to adding any g(r), so the 0.2*f1[r] term is
dropped.  Each core therefore computes (transposed layout: n on partitions,
r on the free dim):
    x[n, r]  = 0.8*relu(f1[r] + f2[n]) + biasT[n, r]        (PE accumulates)
    e[n, r]  = exp(x + 0.2*f2[n])                           (one ACT pass)
    valsT    = [sf | 1s].T @ e   -> [O+1, R] rows 0..O-1 = unnormalized vals,
                                    row O = softmax denominators
    out      = elu(valsT[:O] / valsT[O])

Sharding: rows r are sharded across 8 cores (1024 each).  The host supplies
per-core transposed shards (seqT, biasT) so all device DMA is contiguous.
seq_fts is computed per-shard and AllGather'ed (together with f2) so every
core has the full [N, O] seq_fts for the column dimension.
"""

import sys

sys.path.insert(0, "/opt/trn_rl_repo")

import numpy as np

import concourse.bacc as bacc
import concourse.bass as bass
import concourse.mybir as mybir
import concourse.tile as tile
from concourse.bass_utils import run_bass_kernel_spmd

F32 = mybir.dt.float32
BF16 = mybir.dt.bfloat16
ADD = mybir.AluOpType.add
MAX = mybir.AluOpType.max
MIN = mybir.AluOpType.min
MULT = mybir.AluOpType.mult
EXP = mybir.ActivationFunctionType.Exp
COPY = mybir.ActivationFunctionType.Copy

M = 8          # cores
N = 8192       # nodes (columns of the attention matrix)
R = N // M     # rows per core (1024)
F_IN = 256
O = 64
P = 128        # partitions
NCH = N // P   # n-chunks per core (64)
RB = R // P    # row-blocks per core (8)
H = 512        # matmul free-dim half (PSUM bank limit)

_CACHED = {}


def build_nc(n=N, r=R, bt_bufs=14):
    nch = n // P
    rb = r // P
    hs = [slice(i * H, min((i + 1) * H, r)) for i in range((r + H - 1) // H)]

    nc = bacc.Bacc(
        "TRN2",
        target_bir_lowering=False,
        debug=False,
        enable_asserts=True,
        num_devices=M,
    )

    seqT_d = nc.dram_tensor("seqT", [F_IN, r], F32, kind="ExternalInput")
    biasT_d = nc.dram_tensor("biasT", [n, r], F32, kind="ExternalInput")
    W1_d = nc.dram_tensor("W1", [F_IN, O], F32, kind="ExternalInput")
    a1_d = nc.dram_tensor("a1", [O, 1], F32, kind="ExternalInput")
    a2_d = nc.dram_tensor("a2", [O, 1], F32, kind="ExternalInput")
    b1_d = nc.dram_tensor("b1", [1, 1], F32, kind="ExternalInput")
    b2_d = nc.dram_tensor("b2", [P, 1], F32, kind="ExternalInput")
    id10_d = nc.dram_tensor("id10", [P, P], F32, kind="ExternalInput")
    onesp_d = nc.dram_tensor("onesp", [1, P], F32, kind="ExternalInput")
    oneso_d = nc.dram_tensor("oneso", [1, O], F32, kind="ExternalInput")
    out_d = nc.dram_tensor("out", [O, r], F32, kind="ExternalOutput")

    with tile.TileContext(nc) as tc:
        with (
            tc.tile_pool(name="const", bufs=1) as cp,
            tc.tile_pool(name="bt", bufs=bt_bufs) as btp,
            tc.tile_pool(name="g", bufs=3) as gp,
            tc.tile_pool(name="e", bufs=3) as ep,
            tc.tile_pool(name="small", bufs=2) as smp,
            tc.tile_pool(name="vp", bufs=1, space="PSUM") as vp,
            tc.tile_pool(name="sp", bufs=1, space="PSUM") as sp,
            tc.tile_pool(name="dram", bufs=1, space="DRAM") as dramp,
        ):
            # ---- constants in ----
            id10 = cp.tile([P, P], F32)
            nc.sync.dma_start(id10[:], id10_d.ap())
            onesp = cp.tile([1, P], F32)
            nc.sync.dma_start(onesp[:], onesp_d.ap())
            oneso = cp.tile([1, O], F32)
            nc.sync.dma_start(oneso[:], oneso_d.ap())
            w1a = cp.tile([P, O], F32)
            nc.sync.dma_start(w1a[:], W1_d.ap()[0:P, :])
            w1b = cp.tile([P, O], F32)
            nc.sync.dma_start(w1b[:], W1_d.ap()[P:F_IN, :])
            a1s = cp.tile([O, 1], F32)
            nc.sync.dma_start(a1s[:], a1_d.ap())
            a2s = cp.tile([O, 1], F32)
            nc.sync.dma_start(a2s[:], a2_d.ap())
            b1s = cp.tile([1, 1], F32)
            nc.sync.dma_start(b1s[:], b1_d.ap())
            b2s = cp.tile([P, 1], F32)
            nc.sync.dma_start(b2s[:], b2_d.ap())
            seqta = cp.tile([P, r], F32)
            nc.sync.dma_start(seqta[:], seqT_d.ap()[0:P, :])
            seqtb = cp.tile([P, r], F32)
            nc.sync.dma_start(seqtb[:], seqT_d.ap()[P:F_IN, :])

            # ---- local shard seq_fts^T : [O, r] = W1.T @ seqT ----
            sft_ps = sp.tile([O, r], F32, tag="scratch")
            for sl in hs:
                nc.tensor.matmul(sft_ps[:, sl], w1a[:], seqta[:, sl], start=True, stop=False)
            for sl in hs:
                nc.tensor.matmul(sft_ps[:, sl], w1b[:], seqtb[:, sl], start=False, stop=True)
            sft = cp.tile([O, r], F32)
            nc.scalar.activation(sft[:], sft_ps[:], COPY)

            # ---- f1 (own rows), F1B broadcast tile ----
            f1_ps = sp.tile([1, r], F32, tag="scratch")
            for sl in hs:
                nc.tensor.matmul(f1_ps[:, sl], a1s[:], sft[:, sl], start=True, stop=True)
            f1row = cp.tile([1, r], F32)
            nc.vector.tensor_scalar(f1row[:], f1_ps[:], b1s[:], 0.8, op0=ADD, op1=MULT)

            f1b_ps = sp.tile([P, r], F32, tag="scratch")
            for sl in hs:
                nc.tensor.matmul(f1b_ps[:, sl], onesp[:], f1row[:, sl], start=True, stop=True)
            f1b = cp.tile([P, r], F32)
            nc.scalar.activation(f1b[:], f1b_ps[:], COPY)

            # ---- f2 of own rows: per row-block [O,P] slice of sft, matmul a2 ----
            f2_ps = sp.tile([P, rb], F32, tag="scratch")
            for j in range(rb):
                nc.tensor.matmul(
                    f2_ps[:, j : j + 1],
                    sft[:, j * P : (j + 1) * P],
                    a2s[:],
                    start=True,
                    stop=True,
                )
            f2own = cp.tile([P, rb], F32)
            nc.vector.tensor_scalar(f2own[:], f2_ps[:], b2s[:], None, op0=ADD)

            # ---- sf natural ([r, O]) via PE transposes, into AllGather input ----
            agin = dramp.tile([r, O + 1], F32)
            agout = dramp.tile([n, O + 1], F32)
            for j in range(rb):
                t_ps = sp.tile([P, O], F32, tag="scratch")
                nc.tensor.transpose(t_ps[:], sft[:, j * P : (j + 1) * P], id10[0:O, 0:O])
                t_sb = smp.tile([P, O], F32)
                nc.scalar.activation(t_sb[:], t_ps[:], COPY)
                nc.sync.dma_start(agin[j * P : (j + 1) * P, 0:O], t_sb[:])
            nc.sync.dma_start(
                agin[:].rearrange("(b p) o -> p b o", p=P)[:, :, O : O + 1],
                f2own[:].rearrange("p (b one) -> p b one", one=1),
            )

            nc.gpsimd.collective_compute(
                "AllGather",
                mybir.AluOpType.bypass,
                replica_groups=[list(range(M))],
                ins=[agin[:].opt()],
                outs=[agout[:].opt()],
            )

            # ---- unpack gathered: sfaug (bf16, [P, nch*(O+1)]) and f2col ----
            sfall = cp.tile([P, nch * O], F32)
            nc.sync.dma_start(
                sfall[:].rearrange("p (c o) -> p c o", o=O),
                agout[:].rearrange("(c p) o -> p c o", p=P)[:, :, 0:O],
            )
            sfaug = cp.tile([P, nch * (O + 1)], BF16)
            nc.vector.tensor_copy(
                sfaug[:].rearrange("p (c o) -> p c o", o=O + 1)[:, :, 0:O],
                sfall[:].rearrange("p (c o) -> p c o", o=O),
            )
            nc.vector.memset(
                sfaug[:].rearrange("p (c o) -> p c o", o=O + 1)[:, :, O : O + 1], 1.0
            )
            f2col = cp.tile([P, nch], F32)
            nc.sync.dma_start(
                f2col[:].rearrange("p (c one) -> p c one", one=1),
                agout[:].rearrange("(c p) o -> p c o", p=P)[:, :, O : O + 1],
            )
            f2col02 = cp.tile([P, nch], F32)
            nc.vector.tensor_scalar(f2col02[:], f2col[:], 0.2, None, op0=MULT)
            f2col08 = cp.tile([P, nch], F32)
            nc.vector.tensor_scalar(f2col08[:], f2col[:], 0.8, None, op0=MULT)

            # ---- main loop over n-chunks ----
            vals = vp.tile([O + 1, r], F32)
            saug_r = sfaug[:].rearrange("p (c o) -> p c o", o=O + 1)
            for c in range(nch):
                # x = relu(0.8*f1 + 0.8*f2)  (= 0.8*relu(S)), then DMA-accumulates bias
                x = btp.tile([P, r], F32)
                nc.vector.tensor_scalar(
                    x[:], f1b[:], f2col08[:, c : c + 1], 0.0, op0=ADD, op1=MAX
                )
                nc.gpsimd.dma_start(
                    x[:], biasT_d.ap()[c * P : (c + 1) * P, :], accum_op=ADD
                )

                e = ep.tile([P, r], BF16)
                nc.scalar.activation(e[:], x[:], EXP, bias=f2col02[:, c : c + 1])

                for sl in hs:
                    nc.tensor.matmul(
                        vals[:, sl],
                        saug_r[:, c, :],
                        e[:, sl],
                        start=(c == 0),
                        stop=(c == nch - 1),
                    )

            # ---- epilogue: divide by row sums, elu, out ----
            recip = cp.tile([1, r], F32)
            nc.vector.reciprocal(recip[:], vals[O : O + 1, :])
            rb_ps = sp.tile([O, r], F32, tag="scratch")
            for sl in hs:
                nc.tensor.matmul(rb_ps[:, sl], oneso[:], recip[:, sl], start=True, stop=True)
            vals_sb = cp.tile([O, r], F32)
            nc.scalar.activation(vals_sb[:], vals[0:O, :], COPY)
            vn = cp.tile([O, r], F32)
            nc.vector.tensor_mul(vn[:], vals_sb[:], rb_ps[:])
            # elu(x) = (relu(x) - 1) + exp(min(x, 0))
            p2 = cp.tile([O, r], F32)
            nc.vector.tensor_scalar(p2[:], vn[:], 0.0, -1.0, op0=MAX, op1=ADD)
            mn = cp.tile([O, r], F32)
            nc.vector.tensor_scalar(mn[:], vn[:], 0.0, None, op0=MIN)
            em = cp.tile([O, r], F32)
            nc.scalar.activation(em[:], mn[:], EXP)
            outT = cp.tile([O, r], F32)
            nc.vector.tensor_add(outT[:], p2[:], em[:])
            nc.sync.dma_start(out_d.ap(), outT[:])

    nc.compile()
    return nc


def get_nc():
    if "nc" not in _CACHED:
        _CACHED["nc"] = build_nc()
    return _CACHED["nc"]


def make_in_maps(seq, bias_mat, W1, a1, b1, a2, b2, n=N, r=R):
    m = n // r
    seq2 = np.asarray(seq, dtype=np.float32).reshape(n, F_IN)
    bias2 = np.asarray(bias_mat, dtype=np.float32).reshape(n, n)
    common = {
        "W1": np.asarray(W1, np.float32).reshape(F_IN, O),
        "a1": np.asarray(a1, np.float32).reshape(O, 1),
        "a2": np.asarray(a2, np.float32).reshape(O, 1),
        "b1": np.asarray(b1, np.float32).reshape(1, 1),
        "b2": np.full((P, 1), np.float32(np.asarray(b2).reshape(())), np.float32),
        "id10": np.eye(P, dtype=np.float32),
        "onesp": np.ones((1, P), np.float32),
        "oneso": np.ones((1, O), np.float32),
    }
    in_maps = []
    for i in range(m):
        rows = slice(i * r, (i + 1) * r)
        in_maps.append(
            dict(
                common,
                seqT=np.ascontiguousarray(seq2[rows, :].T),
                biasT=np.ascontiguousarray(bias2[rows, :].T),
            )
        )
    return in_maps


def kernel(seq, bias_mat, W1, a1, b1, a2, b2):
    nc = get_nc()
    in_maps = make_in_maps(seq, bias_mat, W1, a1, b1, a2, b2)
    res = run_bass_kernel_spmd(nc, in_maps, core_ids=list(range(M)))
    outs = [res.results[i]["out"] for i in range(M)]
    full = np.concatenate([o.T for o in outs], axis=0)  # [N, O]
    return full.reshape(1, N, O).astype(np.float32)


if __name__ == "__main__":
    rng = np.random.default_rng(0)
    seq = rng.standard_normal((1, N, F_IN), dtype=np.float32)
    bias = np.zeros((1, N, N), np.float32)
    W1 = (rng.standard_normal((F_IN, O)) * 0.05).astype(np.float32)
    a1 = (rng.standard_normal((O, 1)) * 0.05).astype(np.float32)
    a2 = (rng.standard_normal((O, 1)) * 0.05).astype(np.float32)
    b1 = np.zeros((1,), np.float32)
    b2 = np.zeros((1,), np.float32)
    out = kernel(seq=seq, bias_mat=bias, W1=W1, a1=a1, b1=b1, a2=a2, b2=b2)
    print(out.shape, out.dtype)


# revision 12
# speedup vs baseline: 1.6096x; 1.2549x over previous
"""GAT attention head (single head) distributed across 8 TRN2 NeuronCores.

Math (reference):
    sf   = seq @ W1                        # [N, O]
    f1   = sf @ a1 + b1                    # [N, 1]
    f2   = sf @ a2 + b2                    # [N, 1]
    lg   = f1 + f2.T                       # [N, N]
    co   = softmax(leaky_relu(lg, 0.2) + bias, axis=-1)
    out  = elu(co @ sf)                    # [N, O]

Key algebraic transform: with S = f1[r] + f2[n],
    leaky_relu(S, 0.2) = 0.2*S + 0.8*relu(S)
and softmax over n is invariant to adding any g(r), so the 0.2*f1[r] term is
dropped.  Each core therefore computes (transposed layout: n on partitions,
r on the free dim):
    x[n, r]  = 0.8*relu(f1[r] + f2[n]) + biasT[n, r]        (PE accumulates)
    e[n, r]  = exp(x + 0.2*f2[n])                           (one ACT pass)
    valsT    = [sf | 1s].T @ e   -> [O+1, R] rows 0..O-1 = unnormalized vals,
                                    row O = softmax denominators
    out      = elu(valsT[:O] / valsT[O])

Sharding: rows r are sharded across 8 cores (1024 each).  The host supplies
per-core transposed shards (seqT, biasT) so all device DMA is contiguous.
seq_fts is computed per-shard and AllGather'ed (together with f2) so every
core has the full [N, O] seq_fts for the column dimension.
"""

import sys

sys.path.insert(0, "/opt/trn_rl_repo")

import numpy as np

import concourse.bacc as bacc
import concourse.bass as bass
import concourse.mybir as mybir
import concourse.tile as tile
from concourse.bass_utils import run_bass_kernel_spmd

F32 = mybir.dt.float32
F32R = mybir.dt.float32r
BF16 = mybir.dt.bfloat16
ADD = mybir.AluOpType.add
MAX = mybir.AluOpType.max
MIN = mybir.AluOpType.min
MULT = mybir.AluOpType.mult
EXP = mybir.ActivationFunctionType.Exp
COPY = mybir.ActivationFunctionType.Copy

M = 8          # cores
N = 8192       # nodes (columns of the attention matrix)
R = N // M     # rows per core (1024)
F_IN = 256
O = 64
P = 128        # partitions
NCH = N // P   # n-chunks per core (64)
RB = R // P    # row-blocks per core (8)
H = 512        # matmul free-dim half (PSUM bank limit)

_CACHED = {}


def build_nc(n=N, r=R, bt_bufs=14):
    nch = n // P
    rb = r // P
    hs = [slice(i * H, min((i + 1) * H, r)) for i in range((r + H - 1) // H)]

    nc = bacc.Bacc(
        "TRN2",
        target_bir_lowering=False,
        debug=False,
        enable_asserts=True,
        num_devices=M,
    )

    seqT_d = nc.dram_tensor("seqT", [F_IN, r], F32R, kind="ExternalInput")
    biasT_d = nc.dram_tensor("biasT", [n, r], F32R, kind="ExternalInput")
    W1_d = nc.dram_tensor("W1", [F_IN, O], F32R, kind="ExternalInput")
    a1_d = nc.dram_tensor("a1", [O, 1], F32R, kind="ExternalInput")
    a2_d = nc.dram_tensor("a2", [O, 1], F32R, kind="ExternalInput")
    b1_d = nc.dram_tensor("b1", [1, 1], F32, kind="ExternalInput")
    b2_d = nc.dram_tensor("b2", [P, 1], F32, kind="ExternalInput")
    id10_d = nc.dram_tensor("id10", [P, P], F32R, kind="ExternalInput")
    onesp_d = nc.dram_tensor("onesp", [1, P], F32R, kind="ExternalInput")
    oneso_d = nc.dram_tensor("oneso", [1, O], F32R, kind="ExternalInput")
    out_d = nc.dram_tensor("out", [O, r], F32, kind="ExternalOutput")

    with tile.TileContext(nc) as tc:
        with (
            tc.tile_pool(name="const", bufs=1) as cp,
            tc.tile_pool(name="bt", bufs=bt_bufs) as btp,
            tc.tile_pool(name="g", bufs=3) as gp,
            tc.tile_pool(name="e", bufs=3) as ep,
            tc.tile_pool(name="small", bufs=2) as smp,
            tc.tile_pool(name="xp", bufs=2, space="PSUM") as xp,
            tc.tile_pool(name="vp", bufs=1, space="PSUM") as vp,
            tc.tile_pool(name="sp", bufs=1, space="PSUM") as sp,
            tc.tile_pool(name="dram", bufs=1, space="DRAM") as dramp,
        ):
            # ---- constants in ----
            id10 = cp.tile([P, P], F32R)
            nc.sync.dma_start(id10[:], id10_d.ap())
            onesp = cp.tile([1, P], F32R)
            nc.sync.dma_start(onesp[:], onesp_d.ap())
            oneso = cp.tile([1, O], F32R)
            nc.sync.dma_start(oneso[:], oneso_d.ap())
            w1a = cp.tile([P, O], F32R)
            nc.sync.dma_start(w1a[:], W1_d.ap()[0:P, :])
            w1b = cp.tile([P, O], F32R)
            nc.sync.dma_start(w1b[:], W1_d.ap()[P:F_IN, :])
            a1s = cp.tile([O, 1], F32R)
            nc.sync.dma_start(a1s[:], a1_d.ap())
            a2s = cp.tile([O, 1], F32R)
            nc.sync.dma_start(a2s[:], a2_d.ap())
            b1s = cp.tile([1, 1], F32)
            nc.sync.dma_start(b1s[:], b1_d.ap())
            b2s = cp.tile([P, 1], F32)
            nc.sync.dma_start(b2s[:], b2_d.ap())
            seqta = cp.tile([P, r], F32R)
            nc.sync.dma_start(seqta[:], seqT_d.ap()[0:P, :])
            seqtb = cp.tile([P, r], F32R)
            nc.sync.dma_start(seqtb[:], seqT_d.ap()[P:F_IN, :])

            # ---- local shard seq_fts^T : [O, r] = W1.T @ seqT ----
            sft_ps = sp.tile([O, r], F32, tag="scratch")
            for sl in hs:
                nc.tensor.matmul(sft_ps[:, sl], w1a[:], seqta[:, sl], start=True, stop=False)
            for sl in hs:
                nc.tensor.matmul(sft_ps[:, sl], w1b[:], seqtb[:, sl], start=False, stop=True)
            sft = cp.tile([O, r], F32R)
            nc.scalar.activation(sft[:], sft_ps[:], COPY)

            # ---- f1 (own rows), F1B broadcast tile ----
            f1_ps = sp.tile([1, r], F32, tag="scratch")
            for sl in hs:
                nc.tensor.matmul(f1_ps[:, sl], a1s[:], sft[:, sl], start=True, stop=True)
            f1row = cp.tile([1, r], F32R)
            nc.vector.tensor_scalar(f1row[:], f1_ps[:], b1s[:], 0.8, op0=ADD, op1=MULT)

            f1b_ps = sp.tile([P, r], F32, tag="scratch")
            for sl in hs:
                nc.tensor.matmul(f1b_ps[:, sl], onesp[:], f1row[:, sl], start=True, stop=True)
            f1b = cp.tile([P, r], F32)
            nc.scalar.activation(f1b[:], f1b_ps[:], COPY)

            # ---- f2 of own rows: per row-block [O,P] slice of sft, matmul a2 ----
            f2_ps = sp.tile([P, rb], F32, tag="scratch")
            for j in range(rb):
                nc.tensor.matmul(
                    f2_ps[:, j : j + 1],
                    sft[:, j * P : (j + 1) * P].bitcast(F32),
                    a2s[:].bitcast(F32),
                    start=True,
                    stop=True,
                )
            f2own = cp.tile([P, rb], F32)
            nc.vector.tensor_scalar(f2own[:], f2_ps[:], b2s[:], None, op0=ADD)

            # ---- sf natural ([r, O]) via PE transposes, into AllGather input ----
            agin = dramp.tile([r, O + 1], F32)
            agout = dramp.tile([n, O + 1], F32)
            for j in range(rb):
                t_ps = sp.tile([P, O], F32R, tag="scratch")
                with nc.allow_low_precision(reason="pure data-movement transpose"):
                    nc.tensor.transpose(t_ps[:], sft[:, j * P : (j + 1) * P], id10[0:O, 0:O])
                t_sb = smp.tile([P, O], F32)
                nc.scalar.activation(t_sb[:], t_ps[:], COPY)
                nc.sync.dma_start(agin[j * P : (j + 1) * P, 0:O], t_sb[:])
            nc.sync.dma_start(
                agin[:].rearrange("(b p) o -> p b o", p=P)[:, :, O : O + 1],
                f2own[:].rearrange("p (b one) -> p b one", one=1),
            )

            nc.gpsimd.collective_compute(
                "AllGather",
                mybir.AluOpType.bypass,
                replica_groups=[list(range(M))],
                ins=[agin[:].opt()],
                outs=[agout[:].opt()],
            )

            # ---- unpack gathered: sfaug (bf16, [P, nch*(O+1)]) and f2col ----
            sfall = cp.tile([P, nch * O], F32)
            nc.sync.dma_start(
                sfall[:].rearrange("p (c o) -> p c o", o=O),
                agout[:].rearrange("(c p) o -> p c o", p=P)[:, :, 0:O],
            )
            sfaug = cp.tile([P, nch * (O + 1)], BF16)
            nc.vector.tensor_copy(
                sfaug[:].rearrange("p (c o) -> p c o", o=O + 1)[:, :, 0:O],
                sfall[:].rearrange("p (c o) -> p c o", o=O),
            )
            nc.vector.memset(
                sfaug[:].rearrange("p (c o) -> p c o", o=O + 1)[:, :, O : O + 1], 1.0
            )
            f2col = cp.tile([P, nch], F32)
            nc.sync.dma_start(
                f2col[:].rearrange("p (c one) -> p c one", one=1),
                agout[:].rearrange("(c p) o -> p c o", p=P)[:, :, O : O + 1],
            )
            f2col02 = cp.tile([P, nch], F32)
            nc.vector.tensor_scalar(f2col02[:], f2col[:], 0.2, None, op0=MULT)
            f2col08 = cp.tile([P, nch], F32)
            nc.vector.tensor_scalar(f2col08[:], f2col[:], 0.8, None, op0=MULT)

            # ---- main loop over n-chunks ----
            vals = vp.tile([O + 1, r], F32)
            saug_r = sfaug[:].rearrange("p (c o) -> p c o", o=O + 1)
            for c in range(nch):
                bt = btp.tile([P, r], F32R)
                nc.sync.dma_start(bt[:], biasT_d.ap()[c * P : (c + 1) * P, :])

                # g = relu(0.8*f1 + 0.8*f2) = 0.8*relu(S)
                g = gp.tile([P, r], F32R)
                nc.vector.tensor_scalar(
                    g[:], f1b[:], f2col08[:, c : c + 1], 0.0, op0=ADD, op1=MAX
                )

                # x = g + bt via PE accumulation (float32r streams 1 col/cycle)
                x = xp.tile([P, r], F32)
                for sl in hs:
                    nc.tensor.matmul(
                        x[:, sl], id10[:], g[:, sl],
                        start=True, stop=False,
                    )
                for sl in hs:
                    nc.tensor.matmul(
                        x[:, sl], id10[:], bt[:, sl],
                        start=False, stop=True,
                    )

                e = ep.tile([P, r], BF16)
                nc.scalar.activation(e[:], x[:], EXP, bias=f2col02[:, c : c + 1])

                for sl in hs:
                    nc.tensor.matmul(
                        vals[:, sl],
                        saug_r[:, c, :],
                        e[:, sl],
                        start=(c == 0),
                        stop=(c == nch - 1),
                    )

            # ---- epilogue: divide by row sums, elu, out ----
            recip = cp.tile([1, r], F32R)
            with nc.allow_low_precision(reason="recip feeds f32r matmul broadcast"):
                nc.vector.reciprocal(recip[:], vals[O : O + 1, :])
            rb_ps = sp.tile([O, r], F32, tag="scratch")
            for sl in hs:
                nc.tensor.matmul(rb_ps[:, sl], oneso[:], recip[:, sl], start=True, stop=True)
            vals_sb = cp.tile([O, r], F32)
            nc.scalar.activation(vals_sb[:], vals[0:O, :], COPY)
            vn = cp.tile([O, r], F32)
            nc.vector.tensor_mul(vn[:], vals_sb[:], rb_ps[:])
            # elu(x) = (relu(x) - 1) + exp(min(x, 0))
            p2 = cp.tile([O, r], F32)
            nc.vector.tensor_scalar(p2[:], vn[:], 0.0, -1.0, op0=MAX, op1=ADD)
            mn = cp.tile([O, r], F32)
            nc.vector.tensor_scalar(mn[:], vn[:], 0.0, None, op0=MIN)
            em = cp.tile([O, r], F32)
            nc.scalar.activation(em[:], mn[:], EXP)
            outT = cp.tile([O, r], F32)
            nc.vector.tensor_add(outT[:], p2[:], em[:])
            nc.sync.dma_start(out_d.ap(), outT[:])

    nc.compile()
    return nc


def get_nc():
    if "nc" not in _CACHED:
        _CACHED["nc"] = build_nc()
    return _CACHED["nc"]


def make_in_maps(seq, bias_mat, W1, a1, b1, a2, b2, n=N, r=R):
    m = n // r
    seq2 = np.asarray(seq, dtype=np.float32).reshape(n, F_IN)
    bias2 = np.asarray(bias_mat, dtype=np.float32).reshape(n, n)
    common = {
        "W1": np.asarray(W1, np.float32).reshape(F_IN, O),
        "a1": np.asarray(a1, np.float32).reshape(O, 1),
        "a2": np.asarray(a2, np.float32).reshape(O, 1),
        "b1": np.asarray(b1, np.float32).reshape(1, 1),
        "b2": np.full((P, 1), np.float32(np.asarray(b2).reshape(())), np.float32),
        "id10": np.eye(P, dtype=np.float32),
        "onesp": np.ones((1, P), np.float32),
        "oneso": np.ones((1, O), np.float32),
    }
    in_maps = []
    for i in range(m):
        rows = slice(i * r, (i + 1) * r)
        in_maps.append(
            dict(
                common,
                seqT=np.ascontiguousarray(seq2[rows, :].T),
                biasT=np.ascontiguousarray(bias2[rows, :].T),
            )
        )
    return in_maps


def kernel(seq, bias_mat, W1, a1, b1, a2, b2):
    nc = get_nc()
    in_maps = make_in_maps(seq, bias_mat, W1, a1, b1, a2, b2)
    res = run_bass_kernel_spmd(nc, in_maps, core_ids=list(range(M)))
    outs = [res.results[i]["out"] for i in range(M)]
    full = np.concatenate([o.T for o in outs], axis=0)  # [N, O]
    return full.reshape(1, N, O).astype(np.float32)


if __name__ == "__main__":
    rng = np.random.default_rng(0)
    seq = rng.standard_normal((1, N, F_IN), dtype=np.float32)
    bias = np.zeros((1, N, N), np.float32)
    W1 = (rng.standard_normal((F_IN, O)) * 0.05).astype(np.float32)
    a1 = (rng.standard_normal((O, 1)) * 0.05).astype(np.float32)
    a2 = (rng.standard_normal((O, 1)) * 0.05).astype(np.float32)
    b1 = np.zeros((1,), np.float32)
    b2 = np.zeros((1,), np.float32)
    out = kernel(seq=seq, bias_mat=bias, W1=W1, a1=a1, b1=b1, a2=a2, b2=b2)
    print(out.shape, out.dtype)
